# revision 14
# baseline (speedup 1.0000x reference)
"""Trainium2 Bass kernel for a contrastive hinge loss.

Problem (B=32 splits, L=1024 candidates/split, P=8 positives/split, D=256):
    e = l2norm(sent), q = l2norm(query)
    sim[b,l] = e[b,l] . q[b]
    loss = sum_{b, p in pos_b, j in neg_b} relu(sim[b,j] - sim[b,p] + margin) / total
    total = sum_b |pos_b| * |neg_b|

Strategy (data-parallel over B across 8 cores, 4 splits per core):
  Device (per core), all heavy math on-chip:
    - x tile per split in natural [128 part, 8*256] layout (partition p holds
      candidates l = 8p..8p+7, fully contiguous 8KB DMA per partition).
    - dot[l] = sum_d x[l,d] * qhat[b,d]  via fused DVE scalar_tensor_tensor
    - ssq[l] = sum_d x[l,d]^2            via ACT Square + accum (some on DVE)
    - sim = dot * 1/sqrt(ssq)
    - s_vec[b,j] = sim at positive j     via one-hot matmuls on the PE
      (batched over b; a block-diagonal mask kills the cross-split terms)
    - G[b,j] = sum_{l in ALL} relu(sim[b,l] - s_vec[b,j] + margin)
      computed as one broadcasted add + relu + PE column-sum.
  Host:
    - normalizes queries (32x256, trivial), builds the one-hot PH from pos_idx,
      and finishes: loss = [sum G[b,j over unique positives]
                            - sum_{p,q in pos_b} relu(s_q - s_p + margin)] / total
      using the device-returned s_vec (exact cancellation of pos-as-neg terms).

Handles duplicate pos_idx entries (dedup on host; G is per-(b,j) so duplicate
columns are simply not counted twice).
"""

import numpy as np

B, L, P, D = 32, 1024, 8, 256
NCORES = 8
BL = B // NCORES          # 4 splits per core
U = L // 128              # 8 candidates per partition
MARGIN = 0.01

_CACHED = {}


def _build_nc():
    import concourse.bass as bass
    import concourse.mybir as mybir
    import concourse.tile as tile
    from concourse import bacc

    f32 = mybir.dt.float32
    Alu = mybir.AluOpType
    Act = mybir.ActivationFunctionType

    # Bacc (not raw Bass): its compile() runs generate_event_semaphores, which
    # splits multi-wait instructions — walrus allows 1 sync wait per op.
    nc = bacc.Bacc("TRN2")
    # x[b, p, u*D + d] = sent[core*BL + b, 8p + u, d]  (pure reshape on host)
    x = nc.dram_tensor("x", [BL, 128, U * D], f32, kind="ExternalInput")
    # host-normalized queries
    qh = nc.dram_tensor("qh", [BL, D], f32, kind="ExternalInput")
    # one-hot: ph[p, b, u, j] = 1 if pos_idx[b][j] == 8p + u else 0
    ph = nc.dram_tensor("ph", [128, BL, U, P], f32, kind="ExternalInput")
    # block-diagonal mask: bmask[b, b'*P+j] = 1 if b == b' else 0
    bm = nc.dram_tensor("bm", [BL, BL * P], f32, kind="ExternalInput")
    # out[0, 0:32]  = G[b, j]   (sum over ALL candidates of relu(sim - s_bj + m))
    # out[0, 32:64] = s_vec[b, j]
    out = nc.dram_tensor("out", [1, BL * P * 2], f32, kind="ExternalOutput")

    C = BL * U  # 32 (b, u) columns
    with tile.TileContext(nc) as tc:
        with (
            tc.tile_pool(name="singles", bufs=1) as singles,
            tc.tile_pool(name="xpool", bufs=4) as xpool,
            tc.tile_pool(name="psum", bufs=1, space="PSUM") as psum,
        ):
            # ---- all loads up-front, spread over the three DMA rings ----
            # x loads alternate between the two HWDGE rings (SP + ACT) so they
            # run in parallel; qrep/ph/bm go via SWDGE (gpsimd).
            xts = []
            for b in range(BL):
                xt = xpool.tile([128, U * D], f32, tag=f"xt{b}")
                eng = nc.sync if b % 2 == 0 else nc.scalar
                eng.dma_start(out=xt[:, :], in_=x[b, :, :])
                xts.append(xt)

            qrep = singles.tile([128, BL, D], f32)
            qh_ap = qh[:, :]
            qh_bcast = bass.AP(
                tensor=qh_ap.tensor, offset=qh_ap.offset,
                ap=[[0, 128]] + list(qh_ap.ap),
            )
            nc.gpsimd.dma_start(out=qrep[:, :, :], in_=qh_bcast)

            ph_sb = singles.tile([128, BL, U, P], f32)
            nc.gpsimd.dma_start(out=ph_sb[:, :, :, :], in_=ph[:, :, :, :])
            bm_sb = singles.tile([BL, BL * P], f32)
            nc.gpsimd.dma_start(out=bm_sb[:, :], in_=bm[:, :])

            ones_row = singles.tile([1, 128], f32)  # lhsT for partition-replication
            nc.vector.memset(ones_row[:, :], 1.0)
            ones_col = singles.tile([128, 1], f32)  # lhsT for partition-sum
            nc.vector.memset(ones_col[:, :], 1.0)

            # Warm the ACT table set that covers Sqrt/Square/Relu/Copy so the
            # single PSEUDO_LOAD_ACT_FUNC_SET happens before the hot loop.
            warm = singles.tile([1, 1], f32)
            nc.vector.memset(warm[:, :], 1.0)
            nc.scalar.activation(out=warm[0:1, :], in_=warm[0:1, :], func=Act.Sqrt)

            dot_all = singles.tile([128, C], f32)
            ssq_all = singles.tile([128, C], f32)
            sim_all = singles.tile([128, C], f32)
            junk_dve = singles.tile([128, D], f32)
            junk_act = singles.tile([128, D], f32)
            gs = singles.tile([1, BL * P * 2], f32)

            # ---- per-split heavy passes ----
            for b in range(BL):
                xt = xts[b]
                for u in range(U):
                    c = b * U + u
                    xs = xt[:, u * D:(u + 1) * D]
                    # dot on the DVE (fused mult + free-dim accumulate)
                    nc.vector.scalar_tensor_tensor(
                        out=junk_dve[:, :],
                        in0=xs,
                        scalar=1.0,
                        in1=qrep[:, b, :],
                        op0=Alu.mult,
                        op1=Alu.mult,
                        accum_out=dot_all[:, c:c + 1],
                    )
                    # sum-of-squares: mostly ACT, some on DVE to balance
                    # (DVE accum op ~= 503ns, ACT ~= 746ns measured)
                    if c % 5 != 4:
                        nc.scalar.activation(
                            out=junk_act[:, :],
                            in_=xs,
                            func=Act.Square,
                            accum_out=ssq_all[:, c:c + 1],
                        )
                    else:
                        nc.vector.scalar_tensor_tensor(
                            out=junk_dve[:, :],
                            in0=xs,
                            scalar=1.0,
                            in1=xs,
                            op0=Alu.mult,
                            op1=Alu.mult,
                            accum_out=ssq_all[:, c:c + 1],
                        )
                bsl = slice(b * U, (b + 1) * U)
                # ssq -> 1/sqrt(ssq), then sim = dot * rnorm
                nc.scalar.activation(
                    out=ssq_all[:, bsl], in_=ssq_all[:, bsl], func=Act.Sqrt)
                nc.vector.reciprocal(out=ssq_all[:, bsl], in_=ssq_all[:, bsl])
                nc.vector.tensor_mul(
                    out=sim_all[:, bsl], in0=dot_all[:, bsl], in1=ssq_all[:, bsl])

            # ---- s_vec: 8 batched one-hot matmuls over all splits ----
            # out[b, (b', j)] = sum_p sim[p, b, u] * ph[p, b', u, j]; only the
            # block-diagonal b == b' entries are wanted.
            psum_s4 = psum.tile([BL, BL * P], f32)
            sim_by_u = sim_all[:, :].rearrange("p (b u) -> p u b", b=BL)
            for u in range(U):
                nc.tensor.matmul(
                    psum_s4[:, :],
                    lhsT=sim_by_u[:, u, :],
                    rhs=ph_sb[:, :, u, :],
                    start=(u == 0),
                    stop=(u == U - 1),
                )
            s4_sb = singles.tile([BL, BL * P], f32)
            nc.scalar.copy(out=s4_sb[:, :], in_=psum_s4[:, :])
            s4m = singles.tile([BL, BL * P], f32)
            nc.vector.tensor_mul(out=s4m[:, :], in0=s4_sb[:, :], in1=bm_sb[:, :])
            # collapse the block-diagonal to a single row: s_row[0, (b,j)]
            psum_s1 = psum.tile([1, BL * P], f32)
            nc.tensor.matmul(
                psum_s1[0:1, :], lhsT=ones_col[0:BL, 0:1], rhs=s4m[:, :],
                start=True, stop=True)

            # ---- tail ----
            # s_vec -> host output; ms = margin - s_vec
            nc.scalar.copy(out=gs[0:1, BL * P:], in_=psum_s1[0:1, :])
            ms_row = singles.tile([1, BL * P], f32)
            nc.scalar.activation(
                out=ms_row[0:1, :], in_=psum_s1[0:1, :], func=Act.Copy,
                bias=float(MARGIN), scale=-1.0)
            # replicate ms to all partitions
            psum_msrep = psum.tile([128, BL * P], f32)
            nc.tensor.matmul(
                psum_msrep[:, :], lhsT=ones_row[0:1, :], rhs=ms_row[0:1, :],
                start=True, stop=True)
            msrep = singles.tile([128, BL * P], f32)
            nc.scalar.copy(out=msrep[:, :], in_=psum_msrep[:, :])

            # pair[p, b, u, j] = sim[p, (b,u)] + ms[b, j]
            pair = singles.tile([128, BL, U, P], f32)
            sim_b = (sim_all[:, :]
                     .rearrange("p (b u) -> p b u", b=BL)
                     .unsqueeze(3)
                     .broadcast_to((128, BL, U, P)))
            ms_b = (msrep[:, :]
                    .rearrange("p (b j) -> p b j", b=BL)
                    .unsqueeze(2)
                    .broadcast_to((128, BL, U, P)))
            nc.gpsimd.tensor_add(out=pair[:, :, :, :], in0=sim_b, in1=ms_b)
            rel = singles.tile([128, BL, U, P], f32)
            nc.scalar.activation(out=rel[:, :, :, :], in_=pair[:, :, :, :],
                                 func=Act.Relu)
            # column sums over partitions, then over u
            psum_g = psum.tile([1, BL * U * P], f32)
            nc.tensor.matmul(
                psum_g[0:1, :], lhsT=ones_col[:, 0:1],
                rhs=rel[:, :, :, :].rearrange("p b u j -> p (b u j)"),
                start=True, stop=True)
            gsrc = (psum_g[0:1, :]
                    .rearrange("p (b u j) -> p b j u", b=BL, u=U, j=P))
            nc.vector.tensor_reduce(
                out=gs[0:1, 0:BL * P].rearrange("p (b j) -> p b j", b=BL),
                in_=gsrc,
                axis=mybir.AxisListType.X,
                op=Alu.add,
            )
            nc.sync.dma_start(out=out[0:1, :], in_=gs[0:1, :])

    nc.finalize()
    return nc


def _get_nc():
    if "nc" not in _CACHED:
        _CACHED["nc"] = _build_nc()
    return _CACHED["nc"]


def _host_prep(sent, query, pos_idx):
    """Build per-core input maps."""
    sent = np.ascontiguousarray(sent, dtype=np.float32)
    query = np.asarray(query, dtype=np.float32)
    pos_idx = np.asarray(pos_idx).astype(np.int64)

    qn = np.linalg.norm(query, axis=-1, keepdims=True)
    qhat = (query / np.maximum(qn, 1e-12)).astype(np.float32)

    ph = np.zeros((B, 128, U, P), dtype=np.float32)
    bb = np.repeat(np.arange(B), P)
    ll = pos_idx.reshape(-1)
    jj = np.tile(np.arange(P), B)
    ph[bb, ll // U, ll % U, jj] = 1.0

    bmask = np.zeros((BL, BL * P), dtype=np.float32)
    for b in range(BL):
        bmask[b, b * P:(b + 1) * P] = 1.0

    in_maps = []
    for core in range(NCORES):
        sl = slice(core * BL, (core + 1) * BL)
        in_maps.append({
            "x": sent[sl].reshape(BL, 128, U * D),
            "qh": qhat[sl],
            "ph": np.ascontiguousarray(ph[sl].transpose(1, 0, 2, 3)),
            "bm": bmask,
        })
    return in_maps, pos_idx


def _host_finish(results, pos_idx):
    """Combine per-core (G[b,j], s_vec[b,j]) into the scalar loss."""
    g = np.zeros((B, P), dtype=np.float64)
    s = np.zeros((B, P), dtype=np.float64)
    for core, res in enumerate(results):
        o = res["out"].reshape(2, B // NCORES, P)
        g[core * BL:(core + 1) * BL] = o[0]
        s[core * BL:(core + 1) * BL] = o[1]

    loss = 0.0
    total = 0
    for b in range(B):
        _, first = np.unique(pos_idx[b], return_index=True)
        npos = len(first)
        total += npos * (L - npos)
        sb = s[b, first]
        loss += g[b, first].sum()
        loss -= np.maximum(sb[None, :] - sb[:, None] + MARGIN, 0.0).sum()
    return np.float32(loss / total)


def kernel(sent_embeddings, query_embeddings, pos_idx, splits=None, **_):
    import sys
    if "/opt/trn_rl_repo" not in sys.path:
        sys.path.insert(0, "/opt/trn_rl_repo")
    from concourse.bass_utils import run_bass_kernel_spmd

    in_maps, pos_idx = _host_prep(sent_embeddings, query_embeddings, pos_idx)
    nc = _get_nc()
    res = run_bass_kernel_spmd(nc, in_maps, core_ids=list(range(NCORES)))
    _CACHED["last_result"] = res
    return _host_finish(res.results, pos_idx)


if __name__ == "__main__":
    rng = np.random.default_rng(0)
    sent = rng.standard_normal((B, L, D), dtype=np.float32)
    query = rng.standard_normal((B, D), dtype=np.float32)
    pidx = np.stack([rng.choice(L, P, replace=False) for _ in range(B)])
    print(kernel(sent, query, pidx, L))


# revision 18
# speedup vs baseline: 1.1837x; 1.1837x over previous
"""Trainium2 Bass kernel for a contrastive hinge loss.

Problem (B=32 splits, L=1024 candidates/split, P=8 positives/split, D=256):
    e = l2norm(sent), q = l2norm(query)
    sim[b,l] = e[b,l] . q[b]
    loss = sum_{b, p in pos_b, j in neg_b} relu(sim[b,j] - sim[b,p] + margin) / total
    total = sum_b |pos_b| * |neg_b|

Strategy (data-parallel over B across 8 cores, 4 splits per core):
  Device (per core), all heavy math on-chip:
    - x tile per split in natural [128 part, 8*256] layout (partition p holds
      candidates l = 8p..8p+7, fully contiguous 8KB DMA per partition),
      loads alternating over the two HWDGE rings so they run in parallel.
    - qhat replicated to all partitions via a K=1 matmul on the PE.
    - dot[l] = sum_d x[l,d] * qhat[b,d]  via fused DVE scalar_tensor_tensor
    - ssq[l] = sum_d x[l,d]^2            via ACT Square + accum (some on DVE)
    - sim = dot / sqrt(ssq)              (fused DVE divide)
    - s_vec[b,j] = sim at positive j     via one-hot matmuls on the PE
    - G[b,j] = sum_{l in ALL} relu(sim[b,l] - s_vec[b,j] + margin)
      via broadcasted add + relu (GpSimd) + PE column-sum, per split so the
      tail pipelines under the next split's heavy passes.
  Host:
    - normalizes queries (32x256, trivial), builds the one-hot PH from pos_idx,
      and finishes: loss = [sum G[b,j over unique positives]
                            - sum_{p,q in pos_b} relu(s_q - s_p + margin)] / total
      using the device-returned s_vec (exact cancellation of pos-as-neg terms).

Handles duplicate pos_idx entries (dedup on host; G is per-(b,j) so duplicate
columns are simply not counted twice).
"""

import numpy as np

B, L, P, D = 32, 1024, 8, 256
NCORES = 8
BL = B // NCORES          # 4 splits per core
U = L // 128              # 8 candidates per partition
MARGIN = 0.01

_CACHED = {}


def _build_nc():
    import concourse.bass as bass
    import concourse.mybir as mybir
    import concourse.tile as tile
    from concourse import bacc

    f32 = mybir.dt.float32
    Alu = mybir.AluOpType
    Act = mybir.ActivationFunctionType

    # Bacc (not raw Bass): its compile() runs generate_event_semaphores, which
    # splits multi-wait instructions — walrus allows 1 sync wait per op.
    nc = bacc.Bacc("TRN2")
    # x[b, p, u*D + d] = sent[core*BL + b, 8p + u, d]  (pure reshape on host)
    x = nc.dram_tensor("x", [BL, 128, U * D], f32, kind="ExternalInput")
    # host-normalized queries, concatenated as one row
    qh = nc.dram_tensor("qh", [1, BL * D], f32, kind="ExternalInput")
    # one-hot: ph[p, b, u, j] = 1 if pos_idx[b][j] == 8p + u else 0
    ph = nc.dram_tensor("ph", [128, BL, U, P], f32, kind="ExternalInput")
    # out[0, 0:32]  = G[b, j]   (sum over ALL candidates of relu(sim - s_bj + m))
    # out[0, 32:64] = s_vec[b, j]
    out = nc.dram_tensor("out", [1, BL * P * 2], f32, kind="ExternalOutput")

    C = BL * U  # 32 (b, u) columns
    with tile.TileContext(nc) as tc:
        with (
            tc.tile_pool(name="singles", bufs=1) as singles,
            tc.tile_pool(name="xpool", bufs=4) as xpool,
            tc.tile_pool(name="pp", bufs=1, space="PSUM") as pp,
            tc.tile_pool(name="pstail", bufs=2, space="PSUM") as pstail,
        ):
            # ---- loads up-front, spread over the three DMA rings ----
            xts = []
            for b in range(BL):
                xt = xpool.tile([128, U * D], f32, tag=f"xt{b}")
                eng = nc.sync if b % 2 == 0 else nc.scalar
                eng.dma_start(out=xt[:, :], in_=x[b, :, :])
                xts.append(xt)

            qh_sb = singles.tile([1, BL * D], f32)
            nc.gpsimd.dma_start(out=qh_sb[:, :], in_=qh[:, :])
            ph_sb = singles.tile([128, BL, U, P], f32)
            nc.gpsimd.dma_start(out=ph_sb[:, :, :, :], in_=ph[:, :, :, :])

            ones_row = singles.tile([1, 128], f32)  # lhsT for partition-replication
            nc.vector.memset(ones_row[:, :], 1.0)
            ones_col = singles.tile([128, 1], f32)  # lhsT for partition-sum
            nc.vector.memset(ones_col[:, :], 1.0)
            zeros = singles.tile([128, U * P], f32)  # for relu-by-max on GpSimd
            nc.vector.memset(zeros[:, :], 0.0)

            # Warm the ACT table set covering Sqrt/Square/Relu/Copy before the
            # hot loop (a mid-stream PSEUDO_LOAD_ACT_FUNC_SET costs ~1.3us).
            warm = singles.tile([1, 1], f32)
            nc.vector.memset(warm[:, :], 1.0)
            nc.scalar.activation(out=warm[0:1, :], in_=warm[0:1, :], func=Act.Sqrt)

            # replicate qhat rows to all 128 partitions on the PE
            psum_qrep = pp.tile([128, BL * D], f32)
            for h in range(2):
                nc.tensor.matmul(
                    psum_qrep[:, h * 512:(h + 1) * 512],
                    lhsT=ones_row[0:1, :],
                    rhs=qh_sb[0:1, h * 512:(h + 1) * 512],
                    start=True, stop=True)
            qrep = singles.tile([128, BL, D], f32)
            nc.vector.tensor_copy(
                out=qrep[:, :, :],
                in_=psum_qrep[:, :].rearrange("p (b d) -> p b d", b=BL))

            dot_all = singles.tile([128, C], f32)
            ssq_all = singles.tile([128, C], f32)
            sim_all = singles.tile([128, C], f32)
            # two junk buffers per engine: consecutive same-engine ops then
            # WAW-depend two-back, which Tile sees as already-observed -> no
            # extra event-semaphore per op.
            junk_dve = [singles.tile([128, D], f32, name=f"jd{i}", tag=f"jd{i}")
                        for i in range(2)]
            junk_act = [singles.tile([128, D], f32, name=f"ja{i}", tag=f"ja{i}")
                        for i in range(2)]
            gs = singles.tile([1, BL * P * 2], f32)
            ms_row = singles.tile([1, BL * P], f32)
            msrep = singles.tile([128, BL * P], f32)
            pair = singles.tile([128, BL, U * P], f32)
            rel = singles.tile([128, BL, U * P], f32)

            for b in range(BL):
                xt = xts[b]
                # ---- heavy passes ----
                for u in range(U):
                    c = b * U + u
                    xs = xt[:, u * D:(u + 1) * D]
                    nc.vector.scalar_tensor_tensor(
                        out=junk_dve[u % 2][:, :],
                        in0=xs,
                        scalar=1.0,
                        in1=qrep[:, b, :],
                        op0=Alu.mult,
                        op1=Alu.mult,
                        accum_out=dot_all[:, c:c + 1],
                    )
                    if c % 5 != 4:
                        nc.scalar.activation(
                            out=junk_act[u % 2][:, :],
                            in_=xs,
                            func=Act.Square,
                            accum_out=ssq_all[:, c:c + 1],
                        )
                    else:
                        nc.vector.scalar_tensor_tensor(
                            out=junk_dve[u % 2][:, :],
                            in0=xs,
                            scalar=1.0,
                            in1=xs,
                            op0=Alu.mult,
                            op1=Alu.mult,
                            accum_out=ssq_all[:, c:c + 1],
                        )
                bsl = slice(b * U, (b + 1) * U)
                # sim = dot * 1/sqrt(ssq)
                nc.scalar.activation(
                    out=ssq_all[:, bsl], in_=ssq_all[:, bsl], func=Act.Sqrt)
                nc.vector.reciprocal(out=ssq_all[:, bsl], in_=ssq_all[:, bsl])
                nc.vector.tensor_mul(
                    out=sim_all[:, bsl], in0=dot_all[:, bsl], in1=ssq_all[:, bsl])

                # ---- per-split tail (pipelines under split b+1) ----
                psum_s = pstail.tile([1, P], f32, tag="ps_s")
                for u in range(U):
                    c = b * U + u
                    nc.tensor.matmul(
                        psum_s[0:1, :],
                        lhsT=sim_all[:, c:c + 1],
                        rhs=ph_sb[:, b, u, :],
                        start=(u == 0),
                        stop=(u == U - 1),
                    )
                # s_vec -> host output; ms = margin - s_vec
                nc.scalar.copy(
                    out=gs[0:1, BL * P + b * P:BL * P + (b + 1) * P],
                    in_=psum_s[0:1, :])
                nc.scalar.activation(
                    out=ms_row[0:1, b * P:(b + 1) * P], in_=psum_s[0:1, :],
                    func=Act.Copy, bias=float(MARGIN), scale=-1.0)
                psum_msrep = pstail.tile([128, P], f32, tag="ps_m")
                nc.tensor.matmul(
                    psum_msrep[:, :], lhsT=ones_row[0:1, :],
                    rhs=ms_row[0:1, b * P:(b + 1) * P], start=True, stop=True)
                nc.vector.tensor_copy(
                    out=msrep[:, b * P:(b + 1) * P], in_=psum_msrep[:, :])
                # pair[p, u, j] = sim[p, (b,u)] + ms[b, j]; relu via max(,0)
                sim_b = (sim_all[:, bsl]
                         .unsqueeze(2)
                         .broadcast_to((128, U, P)))
                ms_b = (msrep[:, b * P:(b + 1) * P]
                        .unsqueeze(1)
                        .broadcast_to((128, U, P)))
                nc.gpsimd.tensor_add(
                    out=pair[:, b, :].rearrange("p (u j) -> p u j", u=U),
                    in0=sim_b, in1=ms_b)
                nc.scalar.activation(
                    out=rel[:, b, :], in_=pair[:, b, :], func=Act.Relu)
                psum_g = pstail.tile([1, U * P], f32, tag="ps_g")
                nc.tensor.matmul(
                    psum_g[0:1, :], lhsT=ones_col[:, 0:1], rhs=rel[:, b, :],
                    start=True, stop=True)
                nc.vector.tensor_reduce(
                    out=gs[0:1, b * P:(b + 1) * P]
                        .rearrange("p (o j) -> p o j", o=1),
                    in_=psum_g[0:1, :].rearrange("p (u j) -> p j u", u=U),
                    axis=mybir.AxisListType.X,
                    op=Alu.add,
                )

            nc.sync.dma_start(out=out[0:1, :], in_=gs[0:1, :])

    nc.finalize()
    return nc


def _get_nc():
    if "nc" not in _CACHED:
        _CACHED["nc"] = _build_nc()
    return _CACHED["nc"]


def _host_prep(sent, query, pos_idx):
    """Build per-core input maps."""
    sent = np.ascontiguousarray(sent, dtype=np.float32)
    query = np.asarray(query, dtype=np.float32)
    pos_idx = np.asarray(pos_idx).astype(np.int64)

    qn = np.linalg.norm(query, axis=-1, keepdims=True)
    qhat = (query / np.maximum(qn, 1e-12)).astype(np.float32)

    ph = np.zeros((B, 128, U, P), dtype=np.float32)
    bb = np.repeat(np.arange(B), P)
    ll = pos_idx.reshape(-1)
    jj = np.tile(np.arange(P), B)
    ph[bb, ll // U, ll % U, jj] = 1.0

    in_maps = []
    for core in range(NCORES):
        sl = slice(core * BL, (core + 1) * BL)
        in_maps.append({
            "x": sent[sl].reshape(BL, 128, U * D),
            "qh": qhat[sl].reshape(1, BL * D),
            "ph": np.ascontiguousarray(ph[sl].transpose(1, 0, 2, 3)),
        })
    return in_maps, pos_idx


def _host_finish(results, pos_idx):
    """Combine per-core (G[b,j], s_vec[b,j]) into the scalar loss."""
    g = np.zeros((B, P), dtype=np.float64)
    s = np.zeros((B, P), dtype=np.float64)
    for core, res in enumerate(results):
        o = res["out"].reshape(2, B // NCORES, P)
        g[core * BL:(core + 1) * BL] = o[0]
        s[core * BL:(core + 1) * BL] = o[1]

    loss = 0.0
    total = 0
    for b in range(B):
        _, first = np.unique(pos_idx[b], return_index=True)
        npos = len(first)
        total += npos * (L - npos)
        sb = s[b, first]
        loss += g[b, first].sum()
        loss -= np.maximum(sb[None, :] - sb[:, None] + MARGIN, 0.0).sum()
    return np.float32(loss / total)


def kernel(sent_embeddings, query_embeddings, pos_idx, splits=None, **_):
    import sys
    if "/opt/trn_rl_repo" not in sys.path:
        sys.path.insert(0, "/opt/trn_rl_repo")
    from concourse.bass_utils import run_bass_kernel_spmd

    in_maps, pos_idx = _host_prep(sent_embeddings, query_embeddings, pos_idx)
    nc = _get_nc()
    res = run_bass_kernel_spmd(nc, in_maps, core_ids=list(range(NCORES)))
    _CACHED["last_result"] = res
    return _host_finish(res.results, pos_idx)


if __name__ == "__main__":
    rng = np.random.default_rng(0)
    sent = rng.standard_normal((B, L, D), dtype=np.float32)
    query = rng.standard_normal((B, D), dtype=np.float32)
    pidx = np.stack([rng.choice(L, P, replace=False) for _ in range(B)])
    print(kernel(sent, query, pidx, L))
